# revision 1
# baseline (speedup 1.0000x reference)
"""Multi-head attention (S=2048, B=2, D=1024, H=16) on 8 Trainium2 NeuronCores.

Sharding: tensor-parallel over heads. Each core computes 2 heads end-to-end
(QKV projections restricted to its 128 output dims, attention, and the
row-parallel slice of the output projection). The host sums the 8 partial
outputs (row-parallel Wo ==> partial sums) and adds bo.

On-device compute is fp16 with fp32 PSUM accumulation. The softmax scale and
the (additive log-)mask fold into the exp activation; the softmax denominator
comes for free from a ones-column appended to V.
"""

import math

import numpy as np

S, B, D, H = 2048, 2, 1024, 16
DK = D // H  # 64
NCORES = 8
HLOC = H // NCORES        # heads per core = 2
DLOC = HLOC * DK          # local output dims per core = 128
T = S * B                 # tokens = 4096
KT = D // 128             # contraction tiles = 8
NQC = S // 512            # query chunks per batch = 4
NKB = S // 128            # key blocks = 16
NTT = S // 128            # token tiles per batch = 16
SCALE = 1.0 / math.sqrt(DK)

_prog_cache = {}


def _build(masked: bool):
    import concourse.mybir as mybir
    import concourse.tile as tile
    from concourse import bacc

    f16 = mybir.dt.float16
    f32 = mybir.dt.float32
    EXP = mybir.ActivationFunctionType.Exp
    MUL = mybir.AluOpType.mult
    ADD = mybir.AluOpType.add

    nc = bacc.Bacc("TRN2", target_bir_lowering=False, debug=False)

    def din(name, shape, dt=f16):
        return nc.dram_tensor(name, shape, dt, kind="ExternalInput").ap()

    xq = din("xq", [D, B, S])          # query^T
    xk = din("xk", [D, B, S])          # key^T
    xv = din("xv", [D, B, S])          # value^T
    # per-core projection weights, host-prearranged to [p, kt*m] so the
    # DMA is 128 partitions x 2KB contiguous (wq_arr[p, kt, m] = Wq[hs+m, kt*128+p])
    wq = din("wq", [128, KT * DLOC])
    wk = din("wk", [128, KT * DLOC])
    wv = din("wv", [128, KT * DLOC])
    wo = din("wo", [DLOC, D])          # Wo[:, hs:hs+128].T
    bq = din("bq", [DLOC], f32)
    bk = din("bk", [DLOC], f32)
    bv = din("bv", [DLOC], f32)
    mb = din("mb", [S], f32)           # additive mask bias per key (0 / -1e30)
    out = nc.dram_tensor("out", [S, B, D], f32, kind="ExternalOutput").ap()
    import os
    _dbg = bool(os.environ.get("KDBG"))
    if _dbg:
        dbg_qT = nc.dram_tensor("dbg_qT", [DLOC, B, S], f16, kind="ExternalOutput").ap()
        dbg_kT = nc.dram_tensor("dbg_kT", [DLOC, B, S], f16, kind="ExternalOutput").ap()
        dbg_vv = nc.dram_tensor("dbg_vv", [128, B, HLOC, NKB, 68], f16, kind="ExternalOutput").ap()
        dbg_cn = nc.dram_tensor("dbg_cn", [DLOC, B, S], f16, kind="ExternalOutput").ap()

    with tile.TileContext(nc) as tc:
        with (
            tc.tile_pool(name="wsb", bufs=1) as wsb,
            tc.tile_pool(name="xsb", bufs=8) as xsb,
            tc.tile_pool(name="qkv", bufs=1) as qkv,
            tc.tile_pool(name="esb", bufs=6) as esb,
            tc.tile_pool(name="nrm", bufs=3) as nrm,
            tc.tile_pool(name="osb", bufs=4) as osb,
            tc.tile_pool(name="pj", bufs=2, space="PSUM") as pj,
            tc.tile_pool(name="psc", bufs=2, space="PSUM") as psc,
            tc.tile_pool(name="pcx", bufs=1, space="PSUM") as pcx,
        ):
            # ---- constants / weights -------------------------------------
            w_sb = {}
            for name, ap in (("wq", wq), ("wk", wk), ("wv", wv)):
                t = wsb.tile([128, KT, DLOC], f16, tag=name)
                nc.sync.dma_start(out=t, in_=ap.rearrange("p (kt m) -> p kt m", kt=KT))
                w_sb[name] = t
            wo_sb = wsb.tile([DLOC, D], f16, tag="wo")
            nc.sync.dma_start(out=wo_sb, in_=wo)
            bq_sb = wsb.tile([DLOC, 1], f32, tag="bq")
            nc.sync.dma_start(out=bq_sb, in_=bq.unsqueeze(1))
            bk_sb = wsb.tile([DLOC, 1], f32, tag="bk")
            nc.sync.dma_start(out=bk_sb, in_=bk.unsqueeze(1))
            bv_row = wsb.tile([1, DLOC], f32, tag="bv_row")
            nc.sync.dma_start(out=bv_row, in_=bv.unsqueeze(0))
            bv_bc = wsb.tile([128, DLOC], f32, tag="bv_bc")
            nc.gpsimd.partition_broadcast(bv_bc, bv_row)
            mb_sb = wsb.tile([128, NKB], f32, tag="mb")
            nc.sync.dma_start(out=mb_sb, in_=mb.rearrange("(kb p) -> p kb", p=128))

            # persistent per-batch activations
            qT = [qkv.tile([DLOC, S], f16, tag=f"qT{b}", name=f"qT{b}") for b in range(B)]
            kT = [qkv.tile([DLOC, S], f16, tag=f"kT{b}", name=f"kT{b}") for b in range(B)]
            # V per (head, key-block): [keys=128, 65] with ones in col 64
            vv = [qkv.tile([128, HLOC, NKB, 68], f16, tag=f"vv{b}", name=f"vv{b}") for b in range(B)]
            for b in range(B):
                nc.vector.memset(vv[b], 0.0)
                nc.vector.memset(vv[b][:, :, :, 64:65], 1.0)
            ctxn = [qkv.tile([DLOC, S], f16, tag=f"ctxn{b}", name=f"ctxn{b}") for b in range(B)]

            def load_x(ap, name, b, eng=None):
                eng = eng or nc.sync
                ts = []
                for kt in range(KT):
                    t = xsb.tile([128, S], f16, tag=f"x{name}", name=f"x{name}{kt}")
                    half = S // 2
                    for i in range(2):
                        eng.dma_start(
                            out=t[:, i * half:(i + 1) * half],
                            in_=ap[kt * 128:(kt + 1) * 128, b,
                                   i * half:(i + 1) * half])
                    ts.append(t)
                return ts

            def proj_qk_chunk(b, which, xt, qc):
                w, bias, dst = (("wq", bq_sb, qT) if which == "q"
                                else ("wk", bk_sb, kT))
                ps = pj.tile([DLOC, 512], f32, tag="pj", name="ps")
                sl = slice(qc * 512, (qc + 1) * 512)
                for kt in range(KT):
                    nc.tensor.matmul(ps, w_sb[w][:, kt, :], xt[kt][:, sl],
                                     start=(kt == 0), stop=(kt == KT - 1))
                nc.vector.tensor_scalar(out=dst[b][:, sl], in0=ps,
                                        scalar1=bias, scalar2=None, op0=ADD)

            def proj_qk(b, which, xt):
                for qc in range(NQC):
                    proj_qk_chunk(b, which, xt, qc)

            def proj_v_tt(b, xt, tts):
                for tt in tts:
                    ps = pj.tile([128, DLOC], f32, tag="pj", name="ps")
                    sl = slice(tt * 128, (tt + 1) * 128)
                    for kt in range(KT):
                        nc.tensor.matmul(ps, xt[kt][:, sl], w_sb["wv"][:, kt, :],
                                         start=(kt == 0), stop=(kt == KT - 1))
                    for h in range(HLOC):
                        nc.vector.tensor_tensor(
                            out=vv[b][:, h, tt, 0:64],
                            in0=ps[:, h * 64:(h + 1) * 64],
                            in1=bv_bc[:, h * 64:(h + 1) * 64], op=ADD)

            def outproj_tt(b, tts, on_scalar=False):
                for tt in tts:
                    tsl = slice(tt * 128, (tt + 1) * 128)
                    for eh in range(2):
                        po = pj.tile([128, 512], f32, tag="pj", name="po")
                        nc.tensor.matmul(po, ctxn[b][:, tsl],
                                         wo_sb[:, eh * 512:(eh + 1) * 512],
                                         start=True, stop=True)
                        oc = osb.tile([128, 512], f32, tag="oc", name="oc")
                        if on_scalar:
                            nc.scalar.copy(oc, po)
                        else:
                            nc.vector.tensor_copy(oc, po)
                        nc.gpsimd.dma_start(
                            out=out[tsl, b, eh * 512:(eh + 1) * 512], in_=oc)

            def attn_qc(b, qc, injects=()):
                """Attention for one (b, qc); each inject thunk is emitted
                after a score batch so independent PE work (stage A of the
                next batch, V projections, prior outproj) spreads through
                the stream without starving ACT."""
                injects = list(injects)
                qsl = slice(qc * 512, (qc + 1) * 512)
                pctx = [pcx.tile([65, 512], f32, tag=f"cx{h}", name=f"cx{h}")
                        for h in range(HLOC)]
                escore = {}

                def scores(kbp):
                    psco = [psc.tile([128, 1024], f32, tag="sc", name="sc")
                            for _ in range(HLOC)]
                    for i in range(2):
                        kb = kbp * 2 + i
                        ksl = slice(kb * 128, (kb + 1) * 128)
                        for h in range(HLOC):
                            hsl = slice(h * 64, (h + 1) * 64)
                            nc.tensor.matmul(
                                psco[h][:, i * 512:(i + 1) * 512],
                                kT[b][hsl, ksl], qT[b][hsl, qsl],
                                start=True, stop=True,
                                tile_position=(h * 64, 0))
                    return psco

                def exp_ctx(kbp, psco):
                    for h in range(HLOC):
                        et = esb.tile([128, 1024], f16, tag="e", name="et")
                        if masked:
                            for i in range(2):
                                kb = kbp * 2 + i
                                nc.scalar.activation(
                                    et[:, i * 512:(i + 1) * 512],
                                    psco[h][:, i * 512:(i + 1) * 512],
                                    EXP, bias=mb_sb[:, kb:kb + 1], scale=SCALE)
                        else:
                            nc.scalar.activation(et, psco[h], EXP, scale=SCALE)
                        escore[h] = et
                    for i in range(2):
                        kb = kbp * 2 + i
                        for h in range(HLOC):
                            nc.tensor.matmul(
                                pctx[h], vv[b][:, h, kb, 0:65],
                                escore[h][:, i * 512:(i + 1) * 512],
                                start=(kb == 0), stop=(kb == NKB - 1))

                prev = scores(0)
                cur = scores(1)
                if injects:
                    injects.pop(0)()
                for kbp in range(2, NKB // 2):
                    nxt = scores(kbp)
                    if injects:
                        injects.pop(0)()
                    exp_ctx(kbp - 2, prev)
                    prev, cur = cur, nxt
                exp_ctx(NKB // 2 - 2, prev)
                exp_ctx(NKB // 2 - 1, cur)
                while injects:
                    injects.pop(0)()

                for h in range(HLOC):
                    hsl = slice(h * 64, (h + 1) * 64)
                    cd = nrm.tile([64, 512], f32, tag="cd", name="cd")
                    nc.vector.tensor_copy(cd, pctx[h][0:64, :])
                    cl = nrm.tile([1, 512], f32, tag="cl", name="cl")
                    nc.vector.tensor_copy(cl, pctx[h][64:65, :])
                    # reciprocal_approx_fast requires base partition 0 input
                    rl = nrm.tile([1, 512], f32, tag="rl", name="rl")
                    nc.vector.reciprocal_approx_fast(rl, cl)
                    rl_bc = nrm.tile([64, 512], f32, tag="rlb", name="rlb")
                    nc.gpsimd.partition_broadcast(rl_bc, rl)
                    nc.vector.tensor_tensor(out=ctxn[b][hsl, qsl],
                                            in0=cd, in1=rl_bc, op=MUL)

            def nothing():
                pass

            # stage A head for b=0: K fully + Q chunk 0; Q rest injected.
            xk_t = load_x(xk, "k", 0)
            xq_t = load_x(xq, "q", 0, eng=nc.scalar)
            # PE warmup while the activation DMAs land: junk matmuls on the
            # small weight tiles keep the HAM clock at 2.4GHz.
            for wu in range(40):
                jp = psc.tile([128, 512], f32, tag="sc", name="jp")
                nc.tensor.matmul(jp, w_sb["wq"][:, wu % 8, :],
                                 w_sb["wk"][:, (wu % 2) * 4:(wu % 2) * 4 + 4, :],
                                 start=True, stop=True)
            proj_qk(0, "k", xk_t)
            proj_qk_chunk(0, "q", xq_t, 0)
            xv_t = load_x(xv, "v", 0)

            x2 = {}
            attn_qc(0, 0, [
                lambda: (proj_qk_chunk(0, "q", xq_t, 1),
                         proj_v_tt(0, xv_t, range(0, 4))),
                lambda: proj_v_tt(0, xv_t, range(4, 8)),
                lambda: (proj_qk_chunk(0, "q", xq_t, 2),
                         proj_v_tt(0, xv_t, range(8, 12))),
                lambda: proj_v_tt(0, xv_t, range(12, 16)),
                lambda: proj_qk_chunk(0, "q", xq_t, 3),
            ])
            attn_qc(0, 1, [
                lambda: x2.update(k=load_x(xk, "k", 1)),
                lambda: proj_qk_chunk(1, "k", x2["k"], 0),
                lambda: proj_qk_chunk(1, "k", x2["k"], 1),
                lambda: proj_qk_chunk(1, "k", x2["k"], 2),
                lambda: proj_qk_chunk(1, "k", x2["k"], 3),
            ])
            attn_qc(0, 2, [
                lambda: x2.update(q=load_x(xq, "q", 1)),
                lambda: proj_qk_chunk(1, "q", x2["q"], 0),
                lambda: proj_qk_chunk(1, "q", x2["q"], 1),
                lambda: outproj_tt(0, [0, 1]),
                lambda: proj_qk_chunk(1, "q", x2["q"], 2),
                lambda: proj_qk_chunk(1, "q", x2["q"], 3),
            ])
            attn_qc(0, 3, [
                lambda: x2.update(v=load_x(xv, "v", 1)),
                lambda: proj_v_tt(1, x2["v"], range(0, 4)),
                lambda: proj_v_tt(1, x2["v"], range(4, 8)),
                lambda: outproj_tt(0, [2, 3]),
                lambda: proj_v_tt(1, x2["v"], range(8, 12)),
                lambda: proj_v_tt(1, x2["v"], range(12, 16)),
            ])
            attn_qc(1, 0, [
                lambda: outproj_tt(0, [4, 5]),
                lambda: outproj_tt(0, [6, 7]),
                lambda: outproj_tt(0, [8, 9]),
                lambda: outproj_tt(0, [10, 11]),
                lambda: outproj_tt(0, [12, 13]),
                lambda: outproj_tt(0, [14, 15]),
            ])
            attn_qc(1, 1, [
                lambda: outproj_tt(1, [0, 1]),
                lambda: outproj_tt(1, [2, 3]),
            ])
            attn_qc(1, 2, [
                lambda: outproj_tt(1, [4, 5]),
                lambda: outproj_tt(1, [6, 7]),
            ])
            attn_qc(1, 3, [
                lambda: outproj_tt(1, [8, 9]),
                lambda: outproj_tt(1, [10, 11]),
            ])
            outproj_tt(1, [12, 13], on_scalar=True)
            outproj_tt(1, [14, 15], on_scalar=True)
            if _dbg:
                for b in range(B):
                    nc.sync.dma_start(out=dbg_qT[:, b, :], in_=qT[b])
                    nc.sync.dma_start(out=dbg_kT[:, b, :], in_=kT[b])
                    nc.sync.dma_start(out=dbg_vv[:, b], in_=vv[b])
                    nc.sync.dma_start(out=dbg_cn[:, b, :], in_=ctxn[b])


    nc.compile()
    return nc


def _get_prog(masked: bool):
    key = masked
    if key not in _prog_cache:
        _prog_cache[key] = _build(masked)
    return _prog_cache[key]


def kernel(query, key, value, mask, Wq, bq, Wk, bk, Wv, bv, Wo, bo):
    from concourse.bass_utils import run_bass_kernel_spmd

    query = np.asarray(query)
    key = np.asarray(key)
    value = np.asarray(value)
    mask = np.asarray(mask)
    Wq, bq = np.asarray(Wq), np.asarray(bq)
    Wk, bk = np.asarray(Wk), np.asarray(bk)
    Wv, bv = np.asarray(Wv), np.asarray(bv)
    Wo, bo = np.asarray(Wo), np.asarray(bo)

    masked = not bool(mask.all())
    nc = _get_prog(masked)

    def t16(x):  # [S, B, D] -> contiguous [D, B, S] fp16
        return np.ascontiguousarray(x.transpose(2, 1, 0).astype(np.float16))

    def warr(W, hs):  # [128, KT*128]: row p = concat_kt W[hs+m, kt*128+p]
        wt = W[hs:hs + DLOC, :].T.astype(np.float16)       # [kt*128+p, m]
        return np.ascontiguousarray(
            wt.reshape(KT, 128, DLOC).transpose(1, 0, 2).reshape(128, KT * DLOC))

    xq, xk, xv = t16(query), t16(key), t16(value)
    mb = np.where(mask.reshape(S), 0.0, -1e30).astype(np.float32)

    in_maps = []
    for c in range(NCORES):
        hs = c * DLOC
        in_maps.append({
            "xq": xq, "xk": xk, "xv": xv,
            "wq": warr(Wq, hs),
            "wk": warr(Wk, hs),
            "wv": warr(Wv, hs),
            "wo": np.ascontiguousarray(Wo[:, hs:hs + DLOC].T.astype(np.float16)),
            "bq": bq[hs:hs + DLOC].astype(np.float32),
            "bk": bk[hs:hs + DLOC].astype(np.float32),
            "bv": bv[hs:hs + DLOC].astype(np.float32),
            "mb": mb,
        })

    res = run_bass_kernel_spmd(nc, in_maps, core_ids=list(range(NCORES)))
    acc = res.results[0]["out"].astype(np.float64)
    for c in range(1, NCORES):
        acc += res.results[c]["out"]
    acc += bo.astype(np.float64)
    return acc.astype(np.float32)



# revision 5
# speedup vs baseline: 1.0977x; 1.0977x over previous
"""Multi-head attention (S=2048, B=2, D=1024, H=16) on 8 Trainium2 NeuronCores.

Sharding: batch x heads. Core c handles batch c//4 and heads (c%4)*4..+4,
processed as two head-pairs that map onto a pipelined attention loop
(scores row-tiled per head pair, softmax denominator via a ones-column in V,
QKV projections restricted to the core's 256 output dims, row-parallel
output projection accumulated over both pairs in PSUM). The host sums the
4 partial outputs per batch and adds bo.

On-device compute is fp16 with fp32 PSUM accumulation; output partials are
written fp16. x loads stream in column-chunk order across 4 DMA queues so
the first scores matmul can issue ~17us into the run.
"""

import math

import numpy as np

S, B, D, H = 2048, 2, 1024, 16
DK = D // H               # 64
NCORES = 8
HLOC = 4                  # heads per core
NP = 2                    # head pairs per core
DLOC = HLOC * DK          # local output dims per core = 256
KT = D // 128             # contraction tiles = 8
NQC = S // 512            # query chunks = 4
NKB = S // 128            # key blocks = 16
NTT = S // 128            # token tiles = 16
SCALE = 1.0 / math.sqrt(DK)

_prog_cache = {}


def _build(masked: bool):
    import concourse.mybir as mybir
    import concourse.tile as tile
    from concourse import bacc

    f16 = mybir.dt.float16
    f32 = mybir.dt.float32
    EXP = mybir.ActivationFunctionType.Exp
    MUL = mybir.AluOpType.mult
    ADD = mybir.AluOpType.add

    nc = bacc.Bacc("TRN2", target_bir_lowering=False, debug=False)

    def din(name, shape, dt=f16):
        return nc.dram_tensor(name, shape, dt, kind="ExternalInput").ap()

    xq = din("xq", [D, S])             # query^T, this core's batch
    xk = din("xk", [D, S])
    xv = din("xv", [D, S])
    # projection weights prearranged: w_arr[p, kt, m] = W[hs+m, kt*128+p]
    wq = din("wq", [128, KT * DLOC])
    wk = din("wk", [128, KT * DLOC])
    wv = din("wv", [128, KT * DLOC])
    wo = din("wo", [DLOC, D])          # Wo[:, hs:hs+256].T
    bq = din("bq", [DLOC], f32)
    bk = din("bk", [DLOC], f32)
    bv = din("bv", [DLOC], f32)
    mb = din("mb", [S], f32)           # additive mask bias per key (0 / -1e30)
    out = nc.dram_tensor("out", [S, D], f16, kind="ExternalOutput").ap()

    with tile.TileContext(nc) as tc:
        with (
            tc.tile_pool(name="wsb", bufs=1) as wsb,
            tc.tile_pool(name="xsb", bufs=1) as xsb,
            tc.tile_pool(name="qkv", bufs=1) as qkv,
            tc.tile_pool(name="esb", bufs=6) as esb,
            tc.tile_pool(name="nrm", bufs=3) as nrm,
            tc.tile_pool(name="osb", bufs=4) as osb,
            tc.tile_pool(name="pj", bufs=2, space="PSUM") as pj,
            tc.tile_pool(name="psc", bufs=2, space="PSUM") as psc,
            tc.tile_pool(name="pcx", bufs=1, space="PSUM") as pcx,
        ):
            # ---- weight / bias / mask DMAs (small, front of queues) ----
            wk_sb = wsb.tile([128, KT, DLOC], f16, tag="wk")
            nc.sync.dma_start(out=wk_sb, in_=wk.rearrange("p (kt m) -> p kt m", kt=KT))
            wq_sb = wsb.tile([128, KT, DLOC], f16, tag="wq")
            nc.scalar.dma_start(out=wq_sb, in_=wq.rearrange("p (kt m) -> p kt m", kt=KT))
            w_sb = {"wq": wq_sb, "wk": wk_sb}

            # ---- x tiles: xk full tiles; xq/xv written in column chunks ----
            xk_t = [xsb.tile([128, S], f16, tag=f"xk{kt}", name=f"xk{kt}")
                    for kt in range(KT)]
            xq_t = [xsb.tile([128, S], f16, tag=f"xq{kt}", name=f"xq{kt}")
                    for kt in range(KT)]
            xv_t = [xsb.tile([128, S], f16, tag=f"xv{kt}", name=f"xv{kt}")
                    for kt in range(KT)]
            # only sync (HWDGE), scalar (HWDGE), gpsimd (SWDGE) can issue DMA.
            # scalar's queue is kept clear once attention starts (exp lives
            # there); gpsimd's handles xv + late weights + output stores.
            engs = [nc.sync, nc.scalar, nc.gpsimd]
            for kt in range(KT):
                engs[kt % 3].dma_start(out=xk_t[kt],
                                       in_=xk[kt * 128:(kt + 1) * 128, :])

            def load_chunk(ts, ap, c, eng_pair):
                sl = slice(c * 512, (c + 1) * 512)
                for kt in range(KT):
                    eng_pair[kt // 4].dma_start(out=ts[kt][:, sl],
                                                in_=ap[kt * 128:(kt + 1) * 128, sl])

            load_chunk(xq_t, xq, 0, (nc.sync, nc.scalar))
            load_chunk(xv_t, xv, 0, (nc.gpsimd, nc.gpsimd))

            wv_sb = wsb.tile([128, KT, DLOC], f16, tag="wv")
            nc.gpsimd.dma_start(out=wv_sb, in_=wv.rearrange("p (kt m) -> p kt m", kt=KT))
            w_sb["wv"] = wv_sb
            wo_sb = [wsb.tile([128, D], f16, tag=f"wo{p}", name=f"wo{p}")
                     for p in range(NP)]
            for p in range(NP):
                nc.gpsimd.dma_start(out=wo_sb[p], in_=wo[p * 128:(p + 1) * 128, :])
            bq_sb = wsb.tile([128, NP], f32, tag="bq")
            nc.scalar.dma_start(out=bq_sb, in_=bq.rearrange("(pr i) -> i pr", pr=NP))
            bk_sb = wsb.tile([128, NP], f32, tag="bk")
            nc.scalar.dma_start(out=bk_sb, in_=bk.rearrange("(pr i) -> i pr", pr=NP))
            bv_row = wsb.tile([1, DLOC], f32, tag="bv_row")
            nc.scalar.dma_start(out=bv_row, in_=bv.unsqueeze(0))
            bv_bc = wsb.tile([128, DLOC], f32, tag="bv_bc")
            nc.gpsimd.partition_broadcast(bv_bc, bv_row)
            mb_sb = wsb.tile([128, NKB], f32, tag="mb")
            nc.scalar.dma_start(out=mb_sb, in_=mb.rearrange("(kb p) -> p kb", p=128))

            # remaining x chunks, interleaved so earliest-needed land first
            load_chunk(xq_t, xq, 1, (nc.sync, nc.sync))
            load_chunk(xv_t, xv, 1, (nc.gpsimd, nc.gpsimd))
            load_chunk(xq_t, xq, 2, (nc.sync, nc.sync))
            load_chunk(xv_t, xv, 2, (nc.gpsimd, nc.gpsimd))
            load_chunk(xq_t, xq, 3, (nc.sync, nc.sync))
            load_chunk(xv_t, xv, 3, (nc.gpsimd, nc.gpsimd))

            # ---- persistent per-pair activations -------------------------
            qT = [qkv.tile([128, S], f16, tag=f"qT{p}", name=f"qT{p}") for p in range(NP)]
            kT = [qkv.tile([128, S], f16, tag=f"kT{p}", name=f"kT{p}") for p in range(NP)]
            vv = [qkv.tile([128, 2, NKB, 68], f16, tag=f"vv{p}", name=f"vv{p}")
                  for p in range(NP)]
            for p in range(NP):
                nc.vector.memset(vv[p][:, :, :, 64:65], 1.0)
            ctxn = [qkv.tile([128, S], f16, tag=f"ctxn{p}", name=f"ctxn{p}")
                    for p in range(NP)]

            # ---- projections --------------------------------------------
            def proj_qk_kt_outer(p, which, qcs, bank_of):
                """kt-outer K/Q projection for chunks qcs of pair p.
                bank_of[qc] -> (psum_ap, evac_fn)."""
                w, bias, dst = (("wq", bq_sb, qT) if which == "q"
                                else ("wk", bk_sb, kT))
                for kt in range(KT):
                    for qc in qcs:
                        ps = bank_of[qc]
                        nc.tensor.matmul(ps, w_sb[w][:, kt, p * 128:(p + 1) * 128],
                                         xq_t[kt][:, qc * 512:(qc + 1) * 512]
                                         if which == "q" else
                                         xk_t[kt][:, qc * 512:(qc + 1) * 512],
                                         start=(kt == 0), stop=(kt == KT - 1))
                for qc in qcs:
                    sl = slice(qc * 512, (qc + 1) * 512)
                    nc.vector.tensor_scalar(out=dst[p][:, sl], in0=bank_of[qc],
                                            scalar1=bias[:, p:p + 1], scalar2=None,
                                            op0=ADD)

            def proj_qk_chunk(p, which, qc):
                """Single (pair, chunk) projection through the pj pool."""
                w, bias, dst, xt = (("wq", bq_sb, qT, xq_t) if which == "q"
                                    else ("wk", bk_sb, kT, xk_t))
                ps = pj.tile([128, 512], f32, tag="pj", name="ps")
                sl = slice(qc * 512, (qc + 1) * 512)
                for kt in range(KT):
                    nc.tensor.matmul(ps, w_sb[w][:, kt, p * 128:(p + 1) * 128],
                                     xt[kt][:, sl],
                                     start=(kt == 0), stop=(kt == KT - 1))
                nc.vector.tensor_scalar(out=dst[p][:, sl], in0=ps,
                                        scalar1=bias[:, p:p + 1], scalar2=None,
                                        op0=ADD)

            def proj_v_tt(tts):
                """V projection for token tiles tts, both pairs at once."""
                for tt in tts:
                    ps = pj.tile([128, 512], f32, tag="pj", name="ps")
                    sl = slice(tt * 128, (tt + 1) * 128)
                    for kt in range(KT):
                        nc.tensor.matmul(ps[:, 0:DLOC], xv_t[kt][:, sl],
                                         w_sb["wv"][:, kt, :],
                                         start=(kt == 0), stop=(kt == KT - 1))
                    for p in range(NP):
                        for h in range(2):
                            d0 = (p * 2 + h) * 64
                            nc.vector.tensor_tensor(
                                out=vv[p][:, h, tt, 0:64],
                                in0=ps[:, d0:d0 + 64],
                                in1=bv_bc[:, d0:d0 + 64], op=ADD)

            def outproj_tt(tts, on_scalar=False):
                for tt in tts:
                    tsl = slice(tt * 128, (tt + 1) * 128)
                    for eh in range(2):
                        po = pj.tile([128, 512], f32, tag="pj", name="po")
                        esl = slice(eh * 512, (eh + 1) * 512)
                        for p in range(NP):
                            nc.tensor.matmul(po, ctxn[p][:, tsl], wo_sb[p][:, esl],
                                             start=(p == 0), stop=(p == NP - 1))
                        oc = osb.tile([128, 512], f16, tag="oc", name="oc")
                        if on_scalar:
                            nc.scalar.copy(oc, po)
                        else:
                            nc.vector.tensor_copy(oc, po)
                        nc.gpsimd.dma_start(out=out[tsl, esl], in_=oc)

            # ---- attention ----------------------------------------------
            def attn_qc(p, qc, injects=()):
                injects = list(injects)
                qsl = slice(qc * 512, (qc + 1) * 512)
                pctx = [pcx.tile([65, 512], f32, tag=f"cx{h}", name=f"cx{h}")
                        for h in range(2)]
                escore = {}

                def scores(kbp):
                    psco = [psc.tile([128, 1024], f32, tag="sc", name="sc")
                            for _ in range(2)]
                    for i in range(2):
                        kb = kbp * 2 + i
                        ksl = slice(kb * 128, (kb + 1) * 128)
                        for h in range(2):
                            hsl = slice(h * 64, (h + 1) * 64)
                            nc.tensor.matmul(
                                psco[h][:, i * 512:(i + 1) * 512],
                                kT[p][hsl, ksl], qT[p][hsl, qsl],
                                start=True, stop=True,
                                tile_position=(h * 64, 0))
                    return psco

                def exp_ctx(kbp, psco):
                    for h in range(2):
                        et = esb.tile([128, 1024], f16, tag="e", name="et")
                        if masked:
                            for i in range(2):
                                kb = kbp * 2 + i
                                nc.scalar.activation(
                                    et[:, i * 512:(i + 1) * 512],
                                    psco[h][:, i * 512:(i + 1) * 512],
                                    EXP, bias=mb_sb[:, kb:kb + 1], scale=SCALE)
                        else:
                            nc.scalar.activation(et, psco[h], EXP, scale=SCALE)
                        escore[h] = et
                    for i in range(2):
                        kb = kbp * 2 + i
                        for h in range(2):
                            nc.tensor.matmul(
                                pctx[h], vv[p][:, h, kb, 0:65],
                                escore[h][:, i * 512:(i + 1) * 512],
                                start=(kb == 0), stop=(kb == NKB - 1))

                prev = scores(0)
                cur = scores(1)
                if injects:
                    injects.pop(0)()
                for kbp in range(2, NKB // 2):
                    nxt = scores(kbp)
                    if injects:
                        injects.pop(0)()
                    exp_ctx(kbp - 2, prev)
                    prev, cur = cur, nxt
                exp_ctx(NKB // 2 - 2, prev)
                exp_ctx(NKB // 2 - 1, cur)
                while injects:
                    injects.pop(0)()

                for h in range(2):
                    hsl = slice(h * 64, (h + 1) * 64)
                    cl = nrm.tile([1, 512], f32, tag="cl", name="cl")
                    nc.vector.tensor_copy(cl, pctx[h][64:65, :])
                    rl = nrm.tile([1, 512], f32, tag="rl", name="rl")
                    nc.vector.reciprocal_approx_fast(rl, cl)
                    rl_bc = nrm.tile([64, 512], f32, tag="rlb", name="rlb")
                    nc.gpsimd.partition_broadcast(rl_bc, rl)
                    nc.vector.tensor_tensor(out=ctxn[p][hsl, qsl],
                                            in0=pctx[h][0:64, :], in1=rl_bc, op=MUL)

            # ---- prologue: K-proj pair0 (psc banks + pj), Q chunk0 ------
            # K chains for qc0..3 of pair0: qc0/qc1 in the two halves of a
            # psc slot (distinct PSUM banks), qc2/qc3 in pj slots.
            ksc = psc.tile([128, 1024], f32, tag="sc", name="ksc")
            kpj = [pj.tile([128, 512], f32, tag="pj", name="kpj") for _ in range(2)]
            kbank = {0: ksc[:, 0:512], 1: ksc[:, 512:1024], 2: kpj[0], 3: kpj[1]}
            proj_qk_kt_outer(0, "k", [0, 1, 2, 3], kbank)
            proj_qk_chunk(0, "q", 0)

            # ---- attention schedule with injected projection work -------
            attn_qc(0, 0, [
                lambda: proj_v_tt(range(0, 4)),
                lambda: proj_v_tt(range(4, 6)),
                lambda: proj_v_tt(range(6, 9)),
                lambda: (proj_qk_chunk(0, "q", 1), proj_v_tt(range(9, 11))),
                lambda: proj_v_tt(range(11, 13)),
                lambda: proj_v_tt(range(13, 16)),
                lambda: proj_qk_chunk(0, "q", 2),
            ])
            attn_qc(0, 1, [
                lambda: proj_qk_chunk(0, "q", 3),
                lambda: proj_qk_chunk(1, "k", 0),
                lambda: proj_qk_chunk(1, "k", 1),
                lambda: proj_qk_chunk(1, "k", 2),
                lambda: proj_qk_chunk(1, "k", 3),
            ])
            attn_qc(0, 2, [
                lambda: proj_qk_chunk(1, "q", 0),
                lambda: proj_qk_chunk(1, "q", 1),
                lambda: proj_qk_chunk(1, "q", 2),
                lambda: proj_qk_chunk(1, "q", 3),
            ])
            attn_qc(0, 3, [])
            attn_qc(1, 0, [])
            attn_qc(1, 1, [
                lambda: outproj_tt([0, 1]),
                lambda: outproj_tt([2, 3]),
            ])
            attn_qc(1, 2, [
                lambda: outproj_tt([4, 5]),
                lambda: outproj_tt([6, 7]),
            ])
            attn_qc(1, 3, [
                lambda: outproj_tt([8, 9]),
                lambda: outproj_tt([10, 11]),
            ])
            outproj_tt([12, 13], on_scalar=True)
            outproj_tt([14, 15], on_scalar=True)

    nc.compile()
    return nc


def _get_prog(masked: bool):
    key = masked
    if key not in _prog_cache:
        _prog_cache[key] = _build(masked)
    return _prog_cache[key]


def make_in_maps(query, key, value, mask, Wq, bq, Wk, bk, Wv, bv, Wo, bo):
    query = np.asarray(query)
    key = np.asarray(key)
    value = np.asarray(value)
    mask = np.asarray(mask)
    Wq, bq = np.asarray(Wq), np.asarray(bq)
    Wk, bk = np.asarray(Wk), np.asarray(bk)
    Wv, bv = np.asarray(Wv), np.asarray(bv)
    Wo = np.asarray(Wo)

    def t16(x):  # [S, B, D] -> contiguous [D, B, S] fp16
        return np.ascontiguousarray(x.transpose(2, 1, 0).astype(np.float16))

    def warr(W, hs):  # [128, KT*DLOC]: row p = concat_kt W[hs+m, kt*128+p]
        wt = W[hs:hs + DLOC, :].T.astype(np.float16)       # [kt*128+p, m]
        return np.ascontiguousarray(
            wt.reshape(KT, 128, DLOC).transpose(1, 0, 2).reshape(128, KT * DLOC))

    xq3, xk3, xv3 = t16(query), t16(key), t16(value)
    xqb = [np.ascontiguousarray(xq3[:, b, :]) for b in range(B)]
    xkb = [np.ascontiguousarray(xk3[:, b, :]) for b in range(B)]
    xvb = [np.ascontiguousarray(xv3[:, b, :]) for b in range(B)]
    mbias = np.where(mask.reshape(S), 0.0, -1e30).astype(np.float32)

    wqs = [warr(Wq, g * DLOC) for g in range(4)]
    wks = [warr(Wk, g * DLOC) for g in range(4)]
    wvs = [warr(Wv, g * DLOC) for g in range(4)]
    wos = [np.ascontiguousarray(Wo[:, g * DLOC:(g + 1) * DLOC].T.astype(np.float16))
           for g in range(4)]

    in_maps = []
    for c in range(NCORES):
        b, g = c // 4, c % 4
        hs = g * DLOC
        in_maps.append({
            "xq": xqb[b], "xk": xkb[b], "xv": xvb[b],
            "wq": wqs[g], "wk": wks[g], "wv": wvs[g], "wo": wos[g],
            "bq": bq[hs:hs + DLOC].astype(np.float32),
            "bk": bk[hs:hs + DLOC].astype(np.float32),
            "bv": bv[hs:hs + DLOC].astype(np.float32),
            "mb": mbias,
        })
    return in_maps


def kernel(query, key, value, mask, Wq, bq, Wk, bk, Wv, bv, Wo, bo):
    from concourse.bass_utils import run_bass_kernel_spmd

    mask = np.asarray(mask)
    bo = np.asarray(bo)
    masked = not bool(mask.all())
    nc = _get_prog(masked)
    in_maps = make_in_maps(query, key, value, mask, Wq, bq, Wk, bk, Wv, bv, Wo, bo)

    res = run_bass_kernel_spmd(nc, in_maps, core_ids=list(range(NCORES)))
    acc = np.zeros((S, B, D), dtype=np.float64)
    for c in range(NCORES):
        acc[:, c // 4, :] += res.results[c]["out"].astype(np.float64)
    acc += bo.astype(np.float64)
    return acc.astype(np.float32)


# revision 8
# speedup vs baseline: 1.1483x; 1.0461x over previous
"""Multi-head attention (S=2048, B=2, D=1024, H=16) on 8 Trainium2 NeuronCores.

Sharding: batch x heads. Core c handles batch c//4 and heads (c%4)*4..+4,
processed as two head-pairs that map onto a pipelined attention loop
(scores row-tiled per head pair, softmax denominator via a ones-column in V,
QKV projections restricted to the core's 256 output dims, row-parallel
output projection accumulated over both pairs in PSUM). The host sums the
4 partial outputs per batch and adds bo.

On-device compute is fp16 with fp32 PSUM accumulation; output partials are
written fp16. x loads stream in column-chunk order across 4 DMA queues so
the first scores matmul can issue ~17us into the run.
"""

import math

import numpy as np

S, B, D, H = 2048, 2, 1024, 16
DK = D // H               # 64
NCORES = 8
HLOC = 4                  # heads per core
NP = 2                    # head pairs per core
DLOC = HLOC * DK          # local output dims per core = 256
KT = D // 128             # contraction tiles = 8
NQC = S // 512            # query chunks = 4
NKB = S // 128            # key blocks = 16
NTT = S // 128            # token tiles = 16
SCALE = 1.0 / math.sqrt(DK)

_prog_cache = {}


def _build(masked: bool):
    import concourse.mybir as mybir
    import concourse.tile as tile
    from concourse import bacc

    f16 = mybir.dt.float16
    f32 = mybir.dt.float32
    EXP = mybir.ActivationFunctionType.Exp
    MUL = mybir.AluOpType.mult
    ADD = mybir.AluOpType.add

    nc = bacc.Bacc("TRN2", target_bir_lowering=False, debug=False)

    def din(name, shape, dt=f16):
        return nc.dram_tensor(name, shape, dt, kind="ExternalInput").ap()

    xq = din("xq", [D, S])             # query^T, this core's batch
    xk = din("xk", [D, S])
    xv = din("xv", [D, S])
    # projection weights prearranged: w_arr[p, kt, m] = W[hs+m, kt*128+p]
    wq = din("wq", [128, KT * DLOC])
    wk = din("wk", [128, KT * DLOC])
    wv = din("wv", [128, KT * DLOC])
    wo = din("wo", [DLOC, D])          # Wo[:, hs:hs+256].T
    bq = din("bq", [DLOC], f32)
    bk = din("bk", [DLOC], f32)
    bv = din("bv", [DLOC], f32)
    mb = din("mb", [S], f32)           # additive mask bias per key (0 / -1e30)
    out = nc.dram_tensor("out", [S, D], f16, kind="ExternalOutput").ap()

    with tile.TileContext(nc) as tc:
        with (
            tc.tile_pool(name="wsb", bufs=1) as wsb,
            tc.tile_pool(name="xsb", bufs=1) as xsb,
            tc.tile_pool(name="qkv", bufs=1) as qkv,
            tc.tile_pool(name="esb", bufs=6) as esb,
            tc.tile_pool(name="nrm", bufs=3) as nrm,
            tc.tile_pool(name="osb", bufs=4) as osb,
            tc.tile_pool(name="pj", bufs=2, space="PSUM") as pj,
            tc.tile_pool(name="psc", bufs=2, space="PSUM") as psc,
            tc.tile_pool(name="pcx", bufs=1, space="PSUM") as pcx,
        ):
            # ---- weight / bias / mask DMAs (small, front of queues) ----
            wk_sb = wsb.tile([128, KT, DLOC], f16, tag="wk")
            nc.sync.dma_start(out=wk_sb, in_=wk.rearrange("p (kt m) -> p kt m", kt=KT))
            wq_sb = wsb.tile([128, KT, DLOC], f16, tag="wq")
            nc.scalar.dma_start(out=wq_sb, in_=wq.rearrange("p (kt m) -> p kt m", kt=KT))
            w_sb = {"wq": wq_sb, "wk": wk_sb}

            # ---- x tiles: xk full tiles; xq/xv written in column chunks ----
            xk_t = [xsb.tile([128, S], f16, tag=f"xk{kt}", name=f"xk{kt}")
                    for kt in range(KT)]
            xq_t = [xsb.tile([128, S], f16, tag=f"xq{kt}", name=f"xq{kt}")
                    for kt in range(KT)]
            xv_t = [xsb.tile([128, S], f16, tag=f"xv{kt}", name=f"xv{kt}")
                    for kt in range(KT)]
            # only sync (HWDGE), scalar (HWDGE), gpsimd (SWDGE) can issue DMA.
            # Phase 1: wk/wq + all of xk (scores need every key).
            # Phase 2: first query/value column chunks so attention can start.
            # Phase 3: everything else, earliest-needed first. scalar's queue
            # is kept clear once attention starts (exp lives there).
            for kt in range(3):
                nc.sync.dma_start(out=xk_t[kt], in_=xk[kt * 128:(kt + 1) * 128, :])
            for kt in range(3, 6):
                nc.scalar.dma_start(out=xk_t[kt], in_=xk[kt * 128:(kt + 1) * 128, :])
            for kt in range(6, 8):
                nc.gpsimd.dma_start(out=xk_t[kt], in_=xk[kt * 128:(kt + 1) * 128, :])

            def load_chunk(ts, ap, c0, c1, eng_pair):
                sl = slice(c0 * 512, c1 * 512)
                for kt in range(KT):
                    eng_pair[kt // 4].dma_start(out=ts[kt][:, sl],
                                                in_=ap[kt * 128:(kt + 1) * 128, sl])

            load_chunk(xq_t, xq, 0, 1, (nc.sync, nc.scalar))
            load_chunk(xv_t, xv, 0, 1, (nc.gpsimd, nc.gpsimd))

            wv_sb = wsb.tile([128, KT, DLOC], f16, tag="wv")
            nc.gpsimd.dma_start(out=wv_sb, in_=wv.rearrange("p (kt m) -> p kt m", kt=KT))
            w_sb["wv"] = wv_sb
            wo_sb = [wsb.tile([128, D], f16, tag=f"wo{p}", name=f"wo{p}")
                     for p in range(NP)]
            for p in range(NP):
                nc.gpsimd.dma_start(out=wo_sb[p], in_=wo[p * 128:(p + 1) * 128, :])
            bq_sb = wsb.tile([128, NP], f32, tag="bq")
            nc.scalar.dma_start(out=bq_sb, in_=bq.rearrange("(pr i) -> i pr", pr=NP))
            bk_sb = wsb.tile([128, NP], f32, tag="bk")
            nc.scalar.dma_start(out=bk_sb, in_=bk.rearrange("(pr i) -> i pr", pr=NP))
            bv_row = wsb.tile([1, DLOC], f32, tag="bv_row")
            nc.scalar.dma_start(out=bv_row, in_=bv.unsqueeze(0))
            bv_bc = wsb.tile([128, DLOC], f32, tag="bv_bc")
            nc.gpsimd.partition_broadcast(bv_bc, bv_row)
            mb_sb = wsb.tile([128, NKB], f32, tag="mb")
            nc.scalar.dma_start(out=mb_sb, in_=mb.rearrange("(kb p) -> p kb", p=128))

            # remaining x chunks: one big DMA per kt tile
            load_chunk(xq_t, xq, 1, 4, (nc.sync, nc.sync))
            load_chunk(xv_t, xv, 1, 4, (nc.gpsimd, nc.gpsimd))

            # ---- persistent per-pair activations -------------------------
            qT = [qkv.tile([128, S], f16, tag=f"qT{p}", name=f"qT{p}") for p in range(NP)]
            kT = [qkv.tile([128, S], f16, tag=f"kT{p}", name=f"kT{p}") for p in range(NP)]
            vv = [qkv.tile([128, 2, NKB, 68], f16, tag=f"vv{p}", name=f"vv{p}")
                  for p in range(NP)]
            for p in range(NP):
                nc.vector.memset(vv[p][:, :, :, 64:65], 1.0)
            ctxn = [qkv.tile([128, S], f16, tag=f"ctxn{p}", name=f"ctxn{p}")
                    for p in range(NP)]

            # ---- projections --------------------------------------------
            def proj_qk_kt_outer(p, which, qcs, bank_of):
                """kt-outer K/Q projection for chunks qcs of pair p.
                bank_of[qc] -> (psum_ap, evac_fn)."""
                w, bias, dst = (("wq", bq_sb, qT) if which == "q"
                                else ("wk", bk_sb, kT))
                for kt in range(KT):
                    for qc in qcs:
                        ps = bank_of[qc]
                        nc.tensor.matmul(ps, w_sb[w][:, kt, p * 128:(p + 1) * 128],
                                         xq_t[kt][:, qc * 512:(qc + 1) * 512]
                                         if which == "q" else
                                         xk_t[kt][:, qc * 512:(qc + 1) * 512],
                                         start=(kt == 0), stop=(kt == KT - 1))
                for qc in qcs:
                    sl = slice(qc * 512, (qc + 1) * 512)
                    nc.vector.tensor_scalar(out=dst[p][:, sl], in0=bank_of[qc],
                                            scalar1=bias[:, p:p + 1], scalar2=None,
                                            op0=ADD)

            def proj_qk_chunk(p, which, qc):
                """Single (pair, chunk) projection through the pj pool."""
                w, bias, dst, xt = (("wq", bq_sb, qT, xq_t) if which == "q"
                                    else ("wk", bk_sb, kT, xk_t))
                ps = pj.tile([128, 512], f32, tag="pj", name="ps")
                sl = slice(qc * 512, (qc + 1) * 512)
                for kt in range(KT):
                    nc.tensor.matmul(ps, w_sb[w][:, kt, p * 128:(p + 1) * 128],
                                     xt[kt][:, sl],
                                     start=(kt == 0), stop=(kt == KT - 1))
                nc.vector.tensor_scalar(out=dst[p][:, sl], in0=ps,
                                        scalar1=bias[:, p:p + 1], scalar2=None,
                                        op0=ADD)

            def proj_v_tt(tts):
                """V projection for token tiles tts, both pairs at once."""
                for tt in tts:
                    ps = pj.tile([128, 512], f32, tag="pj", name="ps")
                    sl = slice(tt * 128, (tt + 1) * 128)
                    for kt in range(KT):
                        nc.tensor.matmul(ps[:, 0:DLOC], xv_t[kt][:, sl],
                                         w_sb["wv"][:, kt, :],
                                         start=(kt == 0), stop=(kt == KT - 1))
                    for p in range(NP):
                        for h in range(2):
                            d0 = (p * 2 + h) * 64
                            nc.vector.tensor_tensor(
                                out=vv[p][:, h, tt, 0:64],
                                in0=ps[:, d0:d0 + 64],
                                in1=bv_bc[:, d0:d0 + 64], op=ADD)

            def outproj_tt(tts, store_eng=None):
                store_eng = store_eng or nc.sync
                for tt in tts:
                    tsl = slice(tt * 128, (tt + 1) * 128)
                    for eh in range(2):
                        po = pj.tile([128, 512], f32, tag="pj", name="po")
                        esl = slice(eh * 512, (eh + 1) * 512)
                        for p in range(NP):
                            nc.tensor.matmul(po, ctxn[p][:, tsl], wo_sb[p][:, esl],
                                             start=(p == 0), stop=(p == NP - 1))
                        oc = osb.tile([128, 512], f16, tag="oc", name="oc")
                        nc.vector.tensor_copy(oc, po)
                        store_eng.dma_start(out=out[tsl, esl], in_=oc)

            # ---- attention ----------------------------------------------
            def attn_qc(p, qc, injects=()):
                injects = list(injects)
                qsl = slice(qc * 512, (qc + 1) * 512)
                pctx = [pcx.tile([65, 512], f32, tag=f"cx{h}", name=f"cx{h}")
                        for h in range(2)]
                escore = {}

                def scores(kbp):
                    psco = [psc.tile([128, 1024], f32, tag="sc", name="sc")
                            for _ in range(2)]
                    for i in range(2):
                        kb = kbp * 2 + i
                        ksl = slice(kb * 128, (kb + 1) * 128)
                        for h in range(2):
                            hsl = slice(h * 64, (h + 1) * 64)
                            nc.tensor.matmul(
                                psco[h][:, i * 512:(i + 1) * 512],
                                kT[p][hsl, ksl], qT[p][hsl, qsl],
                                start=True, stop=True,
                                tile_position=(h * 64, 0))
                    return psco

                def exp_ctx(kbp, psco):
                    for h in range(2):
                        et = esb.tile([128, 1024], f16, tag="e", name="et")
                        if masked:
                            for i in range(2):
                                kb = kbp * 2 + i
                                nc.scalar.activation(
                                    et[:, i * 512:(i + 1) * 512],
                                    psco[h][:, i * 512:(i + 1) * 512],
                                    EXP, bias=mb_sb[:, kb:kb + 1], scale=SCALE)
                        else:
                            nc.scalar.activation(et, psco[h], EXP, scale=SCALE)
                        escore[h] = et
                    for i in range(2):
                        kb = kbp * 2 + i
                        for h in range(2):
                            nc.tensor.matmul(
                                pctx[h], vv[p][:, h, kb, 0:65],
                                escore[h][:, i * 512:(i + 1) * 512],
                                start=(kb == 0), stop=(kb == NKB - 1))

                prev = scores(0)
                cur = scores(1)
                if injects:
                    injects.pop(0)()
                for kbp in range(2, NKB // 2):
                    nxt = scores(kbp)
                    if injects:
                        injects.pop(0)()
                    exp_ctx(kbp - 2, prev)
                    prev, cur = cur, nxt
                exp_ctx(NKB // 2 - 2, prev)
                exp_ctx(NKB // 2 - 1, cur)
                while injects:
                    injects.pop(0)()

                for h in range(2):
                    hsl = slice(h * 64, (h + 1) * 64)
                    cl = nrm.tile([1, 512], f32, tag="cl", name="cl")
                    nc.vector.tensor_copy(cl, pctx[h][64:65, :])
                    rl = nrm.tile([1, 512], f32, tag="rl", name="rl")
                    nc.vector.reciprocal_approx_fast(rl, cl)
                    rl_bc = nrm.tile([64, 512], f32, tag="rlb", name="rlb")
                    nc.gpsimd.partition_broadcast(rl_bc, rl)
                    nc.vector.tensor_tensor(out=ctxn[p][hsl, qsl],
                                            in0=pctx[h][0:64, :], in1=rl_bc, op=MUL)

            # ---- prologue: K-proj pair0 (psc banks + pj), Q chunk0 ------
            # K chains for qc0..3 of pair0: qc0/qc1 in the two halves of a
            # psc slot (distinct PSUM banks), qc2/qc3 in pj slots.
            ksc = psc.tile([128, 1024], f32, tag="sc", name="ksc")
            kpj = [pj.tile([128, 512], f32, tag="pj", name="kpj") for _ in range(2)]
            kbank = {0: ksc[:, 0:512], 1: ksc[:, 512:1024], 2: kpj[0], 3: kpj[1]}
            proj_qk_kt_outer(0, "k", [0, 1, 2, 3], kbank)
            proj_qk_chunk(0, "q", 0)

            # ---- attention schedule with injected projection work -------
            attn_qc(0, 0, [
                lambda: proj_v_tt(range(0, 4)),
                lambda: proj_v_tt(range(4, 6)),
                lambda: proj_v_tt(range(6, 9)),
                lambda: (proj_qk_chunk(0, "q", 1), proj_v_tt(range(9, 11))),
                lambda: proj_v_tt(range(11, 13)),
                lambda: proj_v_tt(range(13, 16)),
                lambda: proj_qk_chunk(0, "q", 2),
            ])
            attn_qc(0, 1, [
                lambda: proj_qk_chunk(0, "q", 3),
                lambda: proj_qk_chunk(1, "k", 0),
                lambda: proj_qk_chunk(1, "k", 1),
                lambda: proj_qk_chunk(1, "k", 2),
                lambda: proj_qk_chunk(1, "k", 3),
            ])
            attn_qc(0, 2, [
                lambda: proj_qk_chunk(1, "q", 0),
                lambda: proj_qk_chunk(1, "q", 1),
                lambda: proj_qk_chunk(1, "q", 2),
                lambda: proj_qk_chunk(1, "q", 3),
            ])
            attn_qc(0, 3, [])
            attn_qc(1, 0, [])
            attn_qc(1, 1, [
                lambda: outproj_tt([0, 1]),
                lambda: outproj_tt([2, 3]),
            ])
            attn_qc(1, 2, [
                lambda: outproj_tt([4, 5]),
                lambda: outproj_tt([6, 7]),
            ])
            attn_qc(1, 3, [
                lambda: outproj_tt([8, 9]),
                lambda: outproj_tt([10, 11]),
            ])
            outproj_tt([12, 13], store_eng=nc.scalar)
            outproj_tt([14, 15], store_eng=nc.gpsimd)

    nc.compile()
    return nc


def _get_prog(masked: bool):
    key = masked
    if key not in _prog_cache:
        _prog_cache[key] = _build(masked)
    return _prog_cache[key]


def make_in_maps(query, key, value, mask, Wq, bq, Wk, bk, Wv, bv, Wo, bo):
    query = np.asarray(query)
    key = np.asarray(key)
    value = np.asarray(value)
    mask = np.asarray(mask)
    Wq, bq = np.asarray(Wq), np.asarray(bq)
    Wk, bk = np.asarray(Wk), np.asarray(bk)
    Wv, bv = np.asarray(Wv), np.asarray(bv)
    Wo = np.asarray(Wo)

    def t16(x):  # [S, B, D] -> contiguous [D, B, S] fp16
        return np.ascontiguousarray(x.transpose(2, 1, 0).astype(np.float16))

    def warr(W, hs):  # [128, KT*DLOC]: row p = concat_kt W[hs+m, kt*128+p]
        wt = W[hs:hs + DLOC, :].T.astype(np.float16)       # [kt*128+p, m]
        return np.ascontiguousarray(
            wt.reshape(KT, 128, DLOC).transpose(1, 0, 2).reshape(128, KT * DLOC))

    xq3, xk3, xv3 = t16(query), t16(key), t16(value)
    xqb = [np.ascontiguousarray(xq3[:, b, :]) for b in range(B)]
    xkb = [np.ascontiguousarray(xk3[:, b, :]) for b in range(B)]
    xvb = [np.ascontiguousarray(xv3[:, b, :]) for b in range(B)]
    mbias = np.where(mask.reshape(S), 0.0, -1e30).astype(np.float32)

    wqs = [warr(Wq, g * DLOC) for g in range(4)]
    wks = [warr(Wk, g * DLOC) for g in range(4)]
    wvs = [warr(Wv, g * DLOC) for g in range(4)]
    wos = [np.ascontiguousarray(Wo[:, g * DLOC:(g + 1) * DLOC].T.astype(np.float16))
           for g in range(4)]

    in_maps = []
    for c in range(NCORES):
        b, g = c // 4, c % 4
        hs = g * DLOC
        in_maps.append({
            "xq": xqb[b], "xk": xkb[b], "xv": xvb[b],
            "wq": wqs[g], "wk": wks[g], "wv": wvs[g], "wo": wos[g],
            "bq": bq[hs:hs + DLOC].astype(np.float32),
            "bk": bk[hs:hs + DLOC].astype(np.float32),
            "bv": bv[hs:hs + DLOC].astype(np.float32),
            "mb": mbias,
        })
    return in_maps


def kernel(query, key, value, mask, Wq, bq, Wk, bk, Wv, bv, Wo, bo):
    from concourse.bass_utils import run_bass_kernel_spmd

    mask = np.asarray(mask)
    bo = np.asarray(bo)
    masked = not bool(mask.all())
    nc = _get_prog(masked)
    in_maps = make_in_maps(query, key, value, mask, Wq, bq, Wk, bk, Wv, bv, Wo, bo)

    res = run_bass_kernel_spmd(nc, in_maps, core_ids=list(range(NCORES)))
    acc = np.zeros((S, B, D), dtype=np.float64)
    for c in range(NCORES):
        acc[:, c // 4, :] += res.results[c]["out"].astype(np.float64)
    acc += bo.astype(np.float64)
    return acc.astype(np.float32)
